# revision 10
# baseline (speedup 1.0000x reference)
"""DeepSeek-MoE layer on 8 Trainium2 NeuronCores.

Strategy: data-parallel over tokens (512 tokens/core, all weights replicated).
Each core computes the router, the shared SwiGLU expert and all 8 routed
experts (dense grouped GEMM, matching the reference training path) for its
token slice, entirely in a "transposed" layout: features on SBUF partitions,
tokens on the free dimension. This makes every matmul contraction land on the
partition axis with zero on-device transposes of activations (only the tiny
[8, 512] router block is transposed via the PE). Big matmuls run as fp32r
(~1.5e-4 rel err, full PE rate); the router runs in exact fp32 so top-2
selection matches the reference.

No collectives: the host concatenates the 8 per-core [1024, 512] output
slices (transposed back) into the full [2, 2048, 1024] output.
"""

import sys

sys.path.insert(0, "/opt/trn_rl_repo")

import numpy as np

import concourse.bass as bass
import concourse.bacc as bacc
import concourse.mybir as mybir
import concourse.tile as tile
from concourse.bass_utils import run_bass_kernel_spmd
from concourse.masks import make_identity

F32 = mybir.dt.float32
F32R = mybir.dt.float32r
AF = mybir.ActivationFunctionType
ALU = mybir.AluOpType
AX = mybir.AxisListType

P = 128          # partitions
NCORES = 8
B, T, D = 2, 2048, 1024
N = B * T        # 4096 tokens
TOK = N // NCORES  # 512 tokens per core
HS = 2048        # shared expert hidden
HR = 512         # routed expert hidden
E = 8            # experts
KD = D // P      # 8  k-tiles over d
NHS = HS // P    # 16 h_s tiles
NHR = HR // P    # 4  h_r tiles
ND = D // P      # 8  output d tiles
TOP_K = 2
EPS = 1e-9
OUT_SCALE = 1.0 / 3.0  # 1 / (N_SHARED + TOP_K)

HC = 256               # h-chunk (columns of sw1/sw3 loaded per DMA)
NHC = HS // HC         # 8 chunks
HTPC = HC // P         # 2 h-tiles per chunk


def _emit(nc, tc):
    xT = nc.dram_tensor("xT", [D, TOK], F32R, kind="ExternalInput")
    xTf = nc.dram_tensor("xTf", [D, TOK], F32, kind="ExternalInput")
    tembT = nc.dram_tensor("tembT", [D, 1], F32, kind="ExternalInput")
    rwT = nc.dram_tensor("rwT", [D, E], F32, kind="ExternalInput")
    rtwT = nc.dram_tensor("rtwT", [D, E], F32, kind="ExternalInput")
    biasB = nc.dram_tensor("biasB", [P, E], F32, kind="ExternalInput")
    sw1T = nc.dram_tensor("sw1T", [D, HS], F32R, kind="ExternalInput")
    sw3T = nc.dram_tensor("sw3T", [D, HS], F32R, kind="ExternalInput")
    sw2T = nc.dram_tensor("sw2T", [HS, D], F32R, kind="ExternalInput")
    rw1T = nc.dram_tensor("rw1T", [E, D, HR], F32R, kind="ExternalInput")
    rw2T = nc.dram_tensor("rw2T", [E, HR, D], F32R, kind="ExternalInput")
    outT = nc.dram_tensor("outT", [D, TOK], F32, kind="ExternalOutput")
    dbg_logit = nc.dram_tensor("dbg_logit", [E, TOK], F32, kind="ExternalOutput")
    dbg_comb = nc.dram_tensor("dbg_comb", [E, TOK], F32, kind="ExternalOutput")

    # DRAM views with 128-partition tiling
    xT_v = xT[:].rearrange("(k p) t -> p k t", p=P)            # [128, 8, 512]
    xTf_v = xTf[:].rearrange("(k p) t -> p k t", p=P)
    tembT_v = tembT[:].rearrange("(k p) o -> p k o", p=P)      # [128, 8, 1]
    rwT_v = rwT[:].rearrange("(k p) e -> p k e", p=P)          # [128, 8, 8]
    rtwT_v = rtwT[:].rearrange("(k p) e -> p k e", p=P)
    sw1T_v = sw1T[:].rearrange("(k p) h -> p k h", p=P)        # [128, 8, 2048]
    sw3T_v = sw3T[:].rearrange("(k p) h -> p k h", p=P)
    sw2T_v = sw2T[:].rearrange("(k p) d -> p k d", p=P)        # [128, 16, 1024]
    rw1T_v = rw1T[:].rearrange("e (k p) h -> p e k h", p=P)    # [128, 8, 8, 512]
    rw2T_v = rw2T[:].rearrange("e (k p) d -> p e k d", p=P)    # [128, 8, 4, 1024]
    outT_v = outT[:].rearrange("(dt p) t -> dt p t", p=P)      # [8, 128, 512]

    with (
        tc.tile_pool(name="pconst", bufs=1) as pconst,
        tc.tile_pool(name="pact", bufs=1) as pact,
        tc.tile_pool(name="pstream", bufs=3) as pstream,
        tc.tile_pool(name="ptmp", bufs=2) as ptmp,
        tc.tile_pool(name="pout", bufs=2) as pout,
        tc.tile_pool(name="prt", bufs=1) as prt,
        tc.tile_pool(name="ps", bufs=6, space="PSUM") as ps,
    ):
        # ---- constants / resident inputs ----
        xt = pconst.tile([P, KD, TOK], F32R, tag="xt")
        nc.sync.dma_start(xt[:], xT_v)
        ident = pconst.tile([P, P], F32, tag="ident")
        make_identity(nc, ident[:])
        ones1 = pconst.tile([1, P], F32, tag="ones1")
        nc.vector.memset(ones1[:], 1.0)
        biasb = pconst.tile([P, E], F32, tag="biasb")
        nc.sync.dma_start(biasb[:], biasB[:])
        rwt = pconst.tile([P, KD, E], F32, tag="rwt")
        nc.sync.dma_start(rwt[:], rwT_v)
        rtwt = pconst.tile([P, KD, E], F32, tag="rtwt")
        nc.sync.dma_start(rtwt[:], rtwT_v)
        tembt = pconst.tile([P, KD, 1], F32, tag="tembt")
        nc.sync.dma_start(tembt[:], tembT_v)

        # ---- router ----
        # scoresT[e, t] = sum_d x[t, d] * router_w[e, d]  (exact fp32).
        # x stored as f32r is physically rounded (~13-bit mantissa), so the
        # router streams its own true-fp32 copy of x from DRAM.
        ps_sc = ps.tile([E, TOK], F32, tag="ps")
        for k in range(KD):
            xf = ptmp.tile([P, TOK], F32, tag="xf")
            nc.sync.dma_start(xf[:], xTf_v[:, k, :])
            nc.tensor.matmul(ps_sc[:], rwt[:, k, :], xf[:],
                             start=(k == 0), stop=(k == KD - 1))
        # t_bias[e] = sum_d t_emb[d] * router_t_w[e, d]  -> [E, 1]
        ps_tb = ps.tile([E, 1], F32, tag="ps")
        for k in range(KD):
            nc.tensor.matmul(ps_tb[:], rtwt[:, k, :], tembt[:, k, :],
                             start=(k == 0), stop=(k == KD - 1))
        tb_sb = prt.tile([E, 1], F32, tag="tb")
        nc.vector.tensor_copy(tb_sb[:], ps_tb[:])
        # logitT = scoresT + t_bias (selection happens in logit space: it is
        # monotone in sigmoid(s), avoiding LUT-error top-k flips on near-ties)
        logitT = prt.tile([E, TOK], F32, tag="logitT")
        nc.vector.tensor_tensor(logitT[:], ps_sc[:],
                                tb_sb[:].to_broadcast([E, TOK]), ALU.add)
        nc.sync.dma_start(dbg_logit[:], logitT[:])

        # token-major router math per 128-token tile
        combT = prt.tile([E, TOK], F32, tag="combT")
        for m in range(TOK // P):
            tsl = slice(m * P, (m + 1) * P)
            ps_t = ps.tile([P, E], F32, tag="ps")
            nc.tensor.matmul(ps_t[:], logitT[:, tsl], ident[:E, :E],
                             is_transpose=True, start=True, stop=True)
            l_tok = prt.tile([P, E], F32, tag="l_tok")
            nc.vector.tensor_copy(l_tok[:], ps_t[:])
            s_tok = prt.tile([P, E], F32, tag="s_tok")
            nc.scalar.activation(s_tok[:], l_tok[:], AF.Sigmoid)
            sel = prt.tile([P, E], F32, tag="sel")
            nc.vector.tensor_add(sel[:], l_tok[:], biasb[:])
            m8 = prt.tile([P, E], F32, tag="m8")
            nc.vector.max(m8[:], sel[:])
            mask = prt.tile([P, E], F32, tag="mask")
            nc.vector.tensor_tensor(mask[:], sel[:],
                                    m8[:, 1:2].to_broadcast([P, E]), ALU.is_ge)
            sm = prt.tile([P, E], F32, tag="sm")
            nc.vector.tensor_mul(sm[:], s_tok[:], mask[:])
            den = prt.tile([P, 1], F32, tag="den")
            nc.vector.tensor_reduce(den[:], sm[:], axis=AX.X, op=ALU.add)
            nc.vector.tensor_scalar_add(den[:], den[:], EPS)
            rec = prt.tile([P, 1], F32, tag="rec")
            nc.vector.reciprocal(rec[:], den[:])
            comb = prt.tile([P, E], F32, tag="comb")
            nc.vector.scalar_tensor_tensor(
                comb[:], sm[:], OUT_SCALE, rec[:].to_broadcast([P, E]),
                op0=ALU.mult, op1=ALU.mult)
            ps_ct = ps.tile([E, P], F32, tag="ps")
            nc.tensor.matmul(ps_ct[:], comb[:], ident[:],
                             is_transpose=True, start=True, stop=True)
            nc.vector.tensor_copy(combT[:, tsl], ps_ct[:])
        nc.sync.dma_start(dbg_comb[:], combT[:])

        # ---- shared expert stage 1: actT[h, t] = silu(H1)/3 * H3 ----
        actT = pact.tile([P, NHS, TOK], F32R, tag="actT")
        for hc in range(NHC):
            csl = slice(hc * HC, (hc + 1) * HC)
            w1c = pstream.tile([P, KD, HC], F32R, tag="wstream")
            nc.sync.dma_start(w1c[:], sw1T_v[:, :, csl])
            w3c = pstream.tile([P, KD, HC], F32R, tag="wstream")
            nc.sync.dma_start(w3c[:], sw3T_v[:, :, csl])
            for ht in range(HTPC):
                hsl = slice(ht * P, (ht + 1) * P)
                hidx = hc * HTPC + ht
                ph1 = ps.tile([P, TOK], F32, tag="ps")
                for k in range(KD):
                    nc.tensor.matmul(ph1[:], w1c[:, k, hsl], xt[:, k, :],
                                     start=(k == 0), stop=(k == KD - 1))
                ph3 = ps.tile([P, TOK], F32, tag="ps")
                for k in range(KD):
                    nc.tensor.matmul(ph3[:], w3c[:, k, hsl], xt[:, k, :],
                                     start=(k == 0), stop=(k == KD - 1))
                tsil = ptmp.tile([P, TOK], F32, tag="tmp")
                nc.scalar.activation(tsil[:], ph1[:], AF.Silu)
                nc.vector.scalar_tensor_tensor(
                    actT[:, hidx, :], tsil[:], OUT_SCALE, ph3[:],
                    op0=ALU.mult, op1=ALU.mult)

        # ---- routed experts stage 1: G[e*4+ht] = gelu(H_e) * comb[e] ----
        G = pact.tile([P, E * NHR, TOK], F32R, tag="G")
        for e in range(E):
            r1c = pstream.tile([P, KD, HR], F32R, tag="wstream")
            nc.sync.dma_start(r1c[:], rw1T_v[:, e, :, :])
            # broadcast comb[e, :] across 128 partitions via 1-row matmul
            crow = ptmp.tile([1, TOK], F32, tag="crow")
            nc.sync.dma_start(crow[:], combT[e:e + 1, :])
            ps_cb = ps.tile([P, TOK], F32, tag="ps")
            nc.tensor.matmul(ps_cb[:], ones1[:], crow[:],
                             start=True, stop=True)
            cbb = ptmp.tile([P, TOK], F32, tag="cbb")
            nc.vector.tensor_copy(cbb[:], ps_cb[:])
            for ht in range(NHR):
                hsl = slice(ht * P, (ht + 1) * P)
                ph = ps.tile([P, TOK], F32, tag="ps")
                for k in range(KD):
                    nc.tensor.matmul(ph[:], r1c[:, k, hsl], xt[:, k, :],
                                     start=(k == 0), stop=(k == KD - 1))
                tgel = ptmp.tile([P, TOK], F32, tag="tmp")
                nc.scalar.activation(tgel[:], ph[:], AF.Gelu)
                nc.vector.tensor_mul(G[:, e * NHR + ht, :], tgel[:], cbb[:])

        # ---- stage 2: out[dt] = sum_h sw2T actT + sum_e,k rw2T G ----
        for dt in range(ND):
            dsl = slice(dt * P, (dt + 1) * P)
            w2c = pstream.tile([P, NHS, P], F32R, tag="wstream")
            nc.sync.dma_start(w2c[:], sw2T_v[:, :, dsl])
            r2c = pstream.tile([P, E * NHR, P], F32R, tag="wstream")
            nc.sync.dma_start(r2c[:], rw2T_v[:, :, :, dsl].rearrange(
                "p e k d -> p (e k) d"))
            po = ps.tile([P, TOK], F32, tag="ps")
            nmm = NHS + E * NHR
            i = 0
            for k in range(NHS):
                nc.tensor.matmul(po[:], w2c[:, k, :], actT[:, k, :],
                                 start=(i == 0), stop=(i == nmm - 1))
                i += 1
            for k in range(E * NHR):
                nc.tensor.matmul(po[:], r2c[:, k, :], G[:, k, :],
                                 start=(i == 0), stop=(i == nmm - 1))
                i += 1
            ot = pout.tile([P, TOK], F32, tag="ot")
            nc.vector.tensor_copy(ot[:], po[:])
            nc.sync.dma_start(outT_v[dt], ot[:])


def _make_in_maps(inputs):
    x_flat = np.asarray(inputs["x"], np.float32).reshape(N, D)
    t_emb = np.asarray(inputs["t_emb"], np.float32)
    shared_in = {
        "rwT": np.ascontiguousarray(np.asarray(inputs["router_w"], np.float32).T),
        "rtwT": np.ascontiguousarray(np.asarray(inputs["router_t_w"], np.float32).T),
        "biasB": np.ascontiguousarray(np.broadcast_to(
            np.asarray(inputs["router_bias"], np.float32)[None, :], (P, E))),
        "sw1T": np.ascontiguousarray(np.asarray(inputs["sw1"], np.float32).T),
        "sw3T": np.ascontiguousarray(np.asarray(inputs["sw3"], np.float32).T),
        "sw2T": np.ascontiguousarray(np.asarray(inputs["sw2"], np.float32).T),
        "rw1T": np.ascontiguousarray(np.asarray(inputs["rw1"], np.float32).transpose(0, 2, 1)),
        "rw2T": np.ascontiguousarray(np.asarray(inputs["rw2"], np.float32).transpose(0, 2, 1)),
    }
    in_maps = []
    for c in range(NCORES):
        sl = x_flat[c * TOK:(c + 1) * TOK]
        batch = (c * TOK) // T
        m = dict(shared_in)
        m["xT"] = np.ascontiguousarray(sl.T)
        m["xTf"] = m["xT"]
        m["tembT"] = np.ascontiguousarray(t_emb[batch].reshape(D, 1))
        in_maps.append(m)
    return in_maps


_NC_CACHE = None


def _get_nc():
    global _NC_CACHE
    if _NC_CACHE is None:
        nc = bacc.Bacc(None, target_bir_lowering=False)
        with tile.TileContext(nc) as tc:
            _emit(nc, tc)
        nc.finalize()
        _NC_CACHE = nc
    return _NC_CACHE


def kernel(x, t_emb, router_w, router_t_w, router_bias, sw1, sw3, sw2, rw1, rw2):
    x = np.asarray(x, dtype=np.float32)
    t_emb = np.asarray(t_emb, dtype=np.float32)
    router_w = np.asarray(router_w, dtype=np.float32)
    router_t_w = np.asarray(router_t_w, dtype=np.float32)
    router_bias = np.asarray(router_bias, dtype=np.float32)
    sw1 = np.asarray(sw1, dtype=np.float32)
    sw3 = np.asarray(sw3, dtype=np.float32)
    sw2 = np.asarray(sw2, dtype=np.float32)
    rw1 = np.asarray(rw1, dtype=np.float32)
    rw2 = np.asarray(rw2, dtype=np.float32)

    nc = _get_nc()
    in_maps = _make_in_maps(dict(
        x=x, t_emb=t_emb, router_w=router_w, router_t_w=router_t_w,
        router_bias=router_bias, sw1=sw1, sw3=sw3, sw2=sw2, rw1=rw1, rw2=rw2))

    res = run_bass_kernel_spmd(nc, in_maps, list(range(NCORES)))
    outs = [res.results[c]["outT"] for c in range(NCORES)]
    out = np.concatenate([o.T for o in outs], axis=0)
    return np.ascontiguousarray(out.reshape(B, T, D).astype(np.float32))
